# revision 1
# baseline (speedup 1.0000x reference)
"""Trainium2 Bass kernel for ConvexContractionAttention.

Math notes (derived from the reference):
  block(xi, w, b, a, g, beta) with h = xi*softplus(w)+b, h' = h @ qr(a).Q,
  then batch-norm over (B,T) per (d,j) feature reduces to an affine map of
  the centered input channel:
      out[b,t,d,j] = (xi[b,t,d] - mean_d(xi)) * A[d,j] + beta[d,j]
      A[d,j] = u[d,j]*g[d,j] / sqrt(var_d(xi)*u[d,j]^2 + eps_norm)
      u = softplus(w) @ Q          (bias b cancels through the mean)
  With beta == 0 (true for setup_inputs), per channel d:
      p    = xcq * xck
      s_j  = sigmoid(gamma*Aq_j*Ak_j * p)
      out0 = xcv * (sum_j s_j*Av_j) / (sum_j s_j + eps_w)
  followed by one more batch-affine-norm over (B,T) per channel.

Sharding: channel dim d=1024 split 128-per-core across 8 cores (fully
independent per channel; no collectives). On-chip layout: channels on the
128 SBUF partitions, B*T=8192 on the free axis; the host pre-transposes
each core's shard so every DMA is contiguous.
"""

import sys

if "/opt/trn_rl_repo" not in sys.path:
    sys.path.insert(0, "/opt/trn_rl_repo")

import numpy as np

import concourse.bacc as bacc
import concourse.tile as tile
from concourse import mybir
from concourse import bass_utils

B, T, D = 4, 2048, 1024
BT = B * T
N_CORES = 8
DL = D // N_CORES  # 128 channels per core == SBUF partitions
GAMMA = 5.0
EPS_NORM = 1e-5
EPS_W = 1e-8

F32 = mybir.dt.float32
Act = mybir.ActivationFunctionType
Alu = mybir.AluOpType


def _emit_rsqrt(nc, pool, v, n, tag):
    """out = 1/sqrt(v) elementwise on a tiny [DL, n] fp32 tile, DVE-only.

    Bit-trick seed + 3 Newton iterations (~1e-7 rel); avoids the ScalarE
    Sqrt table set so the whole kernel stays on the sigmoid set.
    """
    U32 = mybir.dt.uint32
    # Seed in "value space": float(bits(v)), then MAGIC - bits/2, then back
    # to an int bit pattern via a value-converting copy. fp32 rounding of
    # the bit pattern perturbs the seed by ~1e-5 rel, well inside Newton's
    # convergence basin.
    bitsf = pool.tile([DL, n], F32, name=f"rsq_b_{tag}", tag=f"rsq_b_{tag}")
    nc.vector.tensor_copy(bitsf, v.bitcast(U32))
    nc.vector.tensor_scalar(
        out=bitsf, in0=bitsf, scalar1=-0.5, scalar2=1597463007.0,
        op0=Alu.mult, op1=Alu.add,
    )
    yu = pool.tile([DL, n], U32, name=f"rsq_y_{tag}", tag=f"rsq_y_{tag}")
    nc.vector.tensor_copy(yu, bitsf)
    y = yu.bitcast(F32)
    t = pool.tile([DL, n], F32, name=f"rsq_t_{tag}", tag=f"rsq_t_{tag}")
    for _ in range(3):
        nc.vector.tensor_mul(t, y, y)          # y^2
        nc.vector.tensor_mul(t, t, v)          # v*y^2
        nc.vector.tensor_scalar(
            out=t, in0=t, scalar1=-0.5, scalar2=1.5, op0=Alu.mult, op1=Alu.add,
        )                                      # 1.5 - 0.5*v*y^2
        nc.vector.tensor_mul(y, y, t)
    return y


def _emit_body(nc, tc, dram, F_main=1024, F_fin=1024, variant="base"):
    """Emit one full pass (phases 1-4) of the per-core kernel."""
    import contextlib

    with contextlib.ExitStack() as ctx:
        resident = ctx.enter_context(tc.tile_pool(name="resident", bufs=1))
        consts = ctx.enter_context(tc.tile_pool(name="consts", bufs=1))
        temps = ctx.enter_context(tc.tile_pool(name="temps", bufs=2))
        stage = ctx.enter_context(tc.tile_pool(name="stage", bufs=3))

        chunks = ("q", "k", "v")

        # Tiny parameter DMAs first so constant chains never wait on them.
        u_sb, ug_sb = {}, {}
        for p in chunks:
            u_sb[p] = consts.tile([DL, 3], F32, name=f"u_{p}", tag=f"u_{p}")
            ug_sb[p] = consts.tile([DL, 3], F32, name=f"ug_{p}", tag=f"ug_{p}")
            nc.sync.dma_start(out=u_sb[p], in_=dram["u" + p])
            nc.sync.dma_start(out=ug_sb[p], in_=dram["ug" + p])
        g_out_sb = consts.tile([DL, 1], F32, name="g_out", tag="g_out")
        b_out_sb = consts.tile([DL, 1], F32, name="b_out", tag="b_out")
        nc.sync.dma_start(out=g_out_sb, in_=dram["g_out"])
        nc.sync.dma_start(out=b_out_sb, in_=dram["b_out"])

        # Warm the sigmoid ACT table set during the input DMA (the whole
        # kernel uses only this set: Sigmoid/Identity/Copy/Square).
        warm = consts.tile([DL, 1], F32, name="warm", tag="warm")
        nc.vector.memset(warm, 0.0)
        nc.scalar.activation(warm, warm, Act.Sigmoid)

        # ---- Phase 1: load x shards; per-channel mean/var; per-chunk consts
        # Stats split: DVE bn_stats on the first 3/4, ScalarE accum pair on
        # the last 1/4 (loaded first). Per-chunk constant chains run inline
        # so q/k constants are ready while v is still streaming in.
        NDVE = 12
        CACT = 2048
        x_sb, mv, neg_mu, Amat = {}, {}, {}, {}
        cmat = None
        for p in chunks:
            x_sb[p] = resident.tile([DL, BT], F32, name=f"x_{p}", tag=f"x_{p}")
            for qi, i in enumerate((3, 0, 1, 2)):
                sl = slice(i * (BT // 4), (i + 1) * (BT // 4))
                eng = nc.sync if (qi % 2 == 0) else nc.scalar
                eng.dma_start(out=x_sb[p][:, sl], in_=dram["x" + p][:, sl])
            st = temps.tile([DL, NDVE, 6], F32, name="bnst", tag="bnst")
            for i in range(NDVE):
                nc.vector.bn_stats(
                    out=st[:, i, :], in_=x_sb[p][:, i * 512 : (i + 1) * 512]
                )
            mv1 = consts.tile([DL, 2], F32, name=f"mv1_{p}", tag=f"mv1_{p}")
            nc.vector.bn_aggr(out=mv1, in_=st)
            hstat = consts.tile([DL, 2], F32, name=f"hs_{p}", tag=f"hs_{p}")
            scr = consts.tile([DL, CACT], F32, name=f"scr_{p}", tag="scr")
            hsl = slice(BT - CACT, BT)
            nc.scalar.activation(scr, x_sb[p][:, hsl], Act.Identity,
                                 accum_out=hstat[:, 0:1])
            nc.scalar.activation(scr, x_sb[p][:, hsl], Act.Square,
                                 accum_out=hstat[:, 1:2])
            # combine (n1 = 3*n2): m=(3m1+m2)/4, v=(3v1+v2)/4+(3/16)(m1-m2)^2
            m2 = consts.tile([DL, 1], F32, name=f"m2_{p}", tag=f"m2_{p}")
            nc.vector.tensor_scalar_mul(out=m2, in0=hstat[:, 0:1], scalar1=1.0 / CACT)
            msq2 = consts.tile([DL, 1], F32, name=f"msq2_{p}", tag=f"msq2_{p}")
            nc.vector.tensor_mul(msq2, m2, m2)
            v2 = consts.tile([DL, 1], F32, name=f"v2_{p}", tag=f"v2_{p}")
            nc.vector.scalar_tensor_tensor(
                out=v2, in0=hstat[:, 1:2], scalar=1.0 / CACT, in1=msq2,
                op0=Alu.mult, op1=Alu.subtract,
            )
            mv[p] = consts.tile([DL, 2], F32, name=f"mv_{p}", tag=f"mv_{p}")
            msum = consts.tile([DL, 1], F32, name=f"msum_{p}", tag=f"msum_{p}")
            nc.vector.scalar_tensor_tensor(
                out=msum, in0=mv1[:, 0:1], scalar=3.0, in1=m2,
                op0=Alu.mult, op1=Alu.add,
            )
            nc.vector.tensor_scalar_mul(out=mv[p][:, 0:1], in0=msum, scalar1=0.25)
            dm = consts.tile([DL, 1], F32, name=f"dm_{p}", tag=f"dm_{p}")
            nc.vector.tensor_sub(dm, mv1[:, 0:1], m2)
            dmsq = consts.tile([DL, 1], F32, name=f"dmsq_{p}", tag=f"dmsq_{p}")
            nc.vector.tensor_mul(dmsq, dm, dm)
            vsum = consts.tile([DL, 1], F32, name=f"vsum_{p}", tag=f"vsum_{p}")
            nc.vector.scalar_tensor_tensor(
                out=vsum, in0=mv1[:, 1:2], scalar=3.0, in1=v2,
                op0=Alu.mult, op1=Alu.add,
            )
            nc.vector.tensor_scalar_mul(out=vsum, in0=vsum, scalar1=0.25)
            nc.vector.scalar_tensor_tensor(
                out=mv[p][:, 1:2], in0=dmsq, scalar=3.0 / 16.0, in1=vsum,
                op0=Alu.mult, op1=Alu.add,
            )
            # per-chunk constants: neg mean, A = u*g*rsqrt(var*u^2+eps)
            neg_mu[p] = consts.tile([DL, 1], F32, name=f"negmu_{p}", tag=f"negmu_{p}")
            nc.vector.tensor_scalar_mul(out=neg_mu[p], in0=mv[p][:, 0:1], scalar1=-1.0)
            usq = consts.tile([DL, 3], F32, name=f"usq_{p}", tag=f"usq_{p}")
            nc.vector.tensor_mul(usq, u_sb[p], u_sb[p])
            vterm = consts.tile([DL, 3], F32, name=f"vt_{p}", tag=f"vt_{p}")
            nc.vector.tensor_scalar(
                out=vterm, in0=usq, scalar1=mv[p][:, 1:2], scalar2=EPS_NORM,
                op0=Alu.mult, op1=Alu.add,
            )
            inv = _emit_rsqrt(nc, consts, vterm, 3, f"A{p}")
            Amat[p] = consts.tile([DL, 3], F32, name=f"A_{p}", tag=f"A_{p}")
            nc.vector.tensor_mul(Amat[p], ug_sb[p], inv)
            if p == "k":
                cmat = consts.tile([DL, 3], F32, name="cmat", tag="cmat")
                nc.vector.tensor_mul(cmat, Amat["q"], Amat["k"])
                nc.vector.tensor_scalar_mul(out=cmat, in0=cmat, scalar1=GAMMA)

        # ---- Phase 2: main loop ----
        muq = mv["q"][:, 0:1]
        muv = mv["v"][:, 0:1]
        Av = Amat["v"]
        NT = BT // F_main
        osum = consts.tile([DL, 2 * NT], F32, name="osum", tag="osum")
        nc.vector.memset(osum, 0.0)
        osq = consts.tile([DL, NT], F32, name="osq", tag="osq")

        if variant == "pe":
            out_un = _emit_main_pe(
                nc, tc, ctx, dram, consts, temps, resident, x_sb, mv, neg_mu,
                Amat, cmat, osum, osq, F_main)
        else:
            out_un = _emit_main_base(
                nc, temps, resident, x_sb, muq, muv, neg_mu, cmat, Av,
                osum, osq, F_main, NT)
        _ = out_un

        # ---- Phase 3: final norm constants ----
        sum_o = consts.tile([DL, 1], F32, name="sum_o", tag="sum_o")
        nc.vector.tensor_reduce(sum_o, osum, axis=mybir.AxisListType.X, op=Alu.add)
        sq_o = consts.tile([DL, 1], F32, name="sq_o", tag="sq_o")
        nc.vector.tensor_reduce(sq_o, osq, axis=mybir.AxisListType.X, op=Alu.add)
        mean_o = consts.tile([DL, 1], F32, name="mean_o", tag="mean_o")
        nc.vector.tensor_scalar_mul(out=mean_o, in0=sum_o, scalar1=1.0 / BT)
        msq_o = consts.tile([DL, 1], F32, name="msq_o", tag="msq_o")
        nc.vector.tensor_mul(msq_o, mean_o, mean_o)
        var_o = consts.tile([DL, 1], F32, name="var_o", tag="var_o")
        nc.vector.scalar_tensor_tensor(
            out=var_o, in0=sq_o, scalar=1.0 / BT, in1=msq_o,
            op0=Alu.mult, op1=Alu.subtract,
        )
        nc.vector.tensor_scalar_add(out=var_o, in0=var_o, scalar1=EPS_NORM)
        rs_o = _emit_rsqrt(nc, consts, var_o, 1, "o")
        fs = consts.tile([DL, 1], F32, name="fs", tag="fs")
        nc.vector.tensor_mul(fs, g_out_sb, rs_o)
        fbt = consts.tile([DL, 1], F32, name="fbt", tag="fbt")
        nc.vector.tensor_mul(fbt, mean_o, fs)
        fb = consts.tile([DL, 1], F32, name="fb", tag="fb")
        nc.vector.tensor_sub(fb, b_out_sb, fbt)

        # ---- Phase 4: final affine + store (split across DVE/ScalarE) ----
        RATIO = F_main // F_fin
        for i in range(BT // F_fin):
            sl = slice(i * F_fin, (i + 1) * F_fin)
            src_ap = out_un[i // RATIO][:, (i % RATIO) * F_fin:(i % RATIO + 1) * F_fin]
            stg = stage.tile([DL, F_fin], F32, name="stg", tag="stg")
            if i % 2 == 0:
                nc.vector.tensor_scalar(
                    out=stg, in0=src_ap, scalar1=fs, scalar2=fb,
                    op0=Alu.mult, op1=Alu.add,
                )
            else:
                nc.scalar.activation(stg, src_ap, Act.Identity, bias=fb, scale=fs)
            eng = nc.sync if (i % 2 == 0) else nc.scalar
            eng.dma_start(out=dram["out"][:, sl], in_=stg)


def _emit_main_base(nc, temps, resident, x_sb, muq, muv, neg_mu, cmat, Av,
                    osum, osq, F_main, NT):
        out_un = []
        for i in range(NT):
            sl = slice(i * F_main, (i + 1) * F_main)
            xq_s, xk_s, xv_s = x_sb["q"][:, sl], x_sb["k"][:, sl], x_sb["v"][:, sl]
            ou = resident.tile([DL, F_main], F32, name=f"ou_{i}", tag=f"ou_{i}")
            out_un.append(ou)

            bA = temps.tile([DL, F_main], F32, name="bA", tag="bA")
            bB = temps.tile([DL, F_main], F32, name="bB", tag="bB")
            bC = temps.tile([DL, F_main], F32, name="bC", tag="bC")
            bD = temps.tile([DL, F_main], F32, name="bD", tag="bD")
            bE = temps.tile([DL, F_main], F32, name="bE", tag="bE")
            bF = temps.tile([DL, F_main], F32, name="bF", tag="bF")

            # xck = xk - mu_k    (ScalarE)
            nc.scalar.activation(bA, xk_s, Act.Identity, bias=neg_mu["k"], scale=1.0)
            # p = (xq - mu_q) * xck
            nc.vector.scalar_tensor_tensor(
                out=bB, in0=xq_s, scalar=muq, in1=bA,
                op0=Alu.subtract, op1=Alu.mult,
            )
            # s_j = sigmoid(c_j * p)
            nc.scalar.activation(bC, bB, Act.Sigmoid, scale=cmat[:, 0:1])
            nc.scalar.activation(bD, bB, Act.Sigmoid, scale=cmat[:, 1:2])
            # t_j = Av_j * s_j
            nc.scalar.activation(bE, bC, Act.Copy, scale=Av[:, 0:1])
            nc.scalar.activation(bF, bD, Act.Copy, scale=Av[:, 1:2])
            # den01 = s0 + s1  (into bC, in-place)
            nc.vector.tensor_add(bC, bC, bD)
            # s2 = sigmoid(c2 * p)  (into bB, in-place over p)
            nc.scalar.activation(bB, bB, Act.Sigmoid, scale=cmat[:, 2:3])
            # t2 = Av2 * s2
            nc.scalar.activation(bA, bB, Act.Copy, scale=Av[:, 2:3])
            # den = (s2 + eps_w) + den01
            nc.vector.scalar_tensor_tensor(
                out=bD, in0=bB, scalar=EPS_W, in1=bC, op0=Alu.add, op1=Alu.add
            )
            # r = 1/den
            nc.vector.reciprocal_approx_fast(out=bB, in_=bD)
            # num = t0 + t1 + t2 ; tt = num * r
            nc.vector.tensor_add(bE, bE, bF)
            nc.vector.tensor_add(bE, bE, bA)
            nc.vector.tensor_mul(bE, bE, bB)
            # out_un = (xv - mu_v) * tt  (accum_out -> per-tile sums)
            nc.vector.scalar_tensor_tensor(
                out=ou, in0=xv_s, scalar=muv, in1=bE,
                op0=Alu.subtract, op1=Alu.mult,
                accum_out=osum[:, i:i + 1],
            )
            # sum of squares of out_un tile (ScalarE; main output is scratch)
            bG = temps.tile([DL, F_main], mybir.dt.bfloat16, name="bG", tag="bG")
            nc.scalar.activation(bG, ou, Act.Square,
                                 accum_out=osq[:, i:i + 1])
        return out_un


def _emit_main_pe(nc, tc, ctx, dram, consts, temps, resident, x_sb, mv, neg_mu,
                  Amat, cmat, osum, osq, F_main):
    """Main loop with num/den sums done on TensorE via diagonal float32r
    matmuls accumulating in PSUM.  DVE: p, eps-add, recip, tt, out per block.
    """
    F32R = mybir.dt.float32r
    FB = 512                      # PSUM block (1 bank)
    NB = F_main // FB
    NT = BT // F_main
    muq = mv["q"][:, 0:1]
    muv = mv["v"][:, 0:1]
    Av = Amat["v"]

    psum = ctx.enter_context(tc.tile_pool(name="psum", bufs=2, space="PSUM"))

    # Stationary matrices: identity (for den) and diag(Av_j) (for num).
    # float32r operands must be produced pre-rounded, so the tiles are
    # declared float32r (same 4-byte layout; 0/1 are exact).
    ident_sb = consts.tile([DL, DL], F32R, name="ident", tag="ident")
    nc.sync.dma_start(out=ident_sb, in_=dram["ident"].bitcast(F32R))
    dg = []
    for j in range(3):
        d = consts.tile([DL, DL], F32R, name=f"dg{j}", tag=f"dg{j}")
        nc.vector.tensor_scalar_mul(out=d, in0=ident_sb, scalar1=Av[:, j:j + 1])
        dg.append(d)

    out_un = []
    for i in range(NT):
        sl = slice(i * F_main, (i + 1) * F_main)
        xq_s, xk_s = x_sb["q"][:, sl], x_sb["k"][:, sl]
        ou = resident.tile([DL, F_main], F32, name=f"ou_{i}", tag=f"ou_{i}")
        out_un.append(ou)

        bP = temps.tile([DL, F_main], F32, name="bP", tag="bP")
        bA = temps.tile([DL, F_main], F32R, name="bA", tag="bA")
        bB = temps.tile([DL, F_main], F32R, name="bB", tag="bB")
        bC = temps.tile([DL, F_main], F32R, name="bC", tag="bC")

        # xck = xk - mu_k (ScalarE);  p = (xq-mu_q)*xck (DVE, in-place)
        nc.scalar.activation(bP, xk_s, Act.Identity, bias=neg_mu["k"], scale=1.0)
        nc.vector.scalar_tensor_tensor(
            out=bP, in0=xq_s, scalar=muq, in1=bP,
            op0=Alu.subtract, op1=Alu.mult,
        )
        # sigmoids (written pre-rounded to float32r for the PE)
        nc.scalar.activation(bC, bP, Act.Sigmoid, scale=cmat[:, 0:1])
        nc.scalar.activation(bA, bP, Act.Sigmoid, scale=cmat[:, 1:2])
        nc.scalar.activation(bB, bP, Act.Sigmoid, scale=cmat[:, 2:3])
        sig = (bC, bA, bB)

        for b in range(NB):
            bsl = slice(b * FB, (b + 1) * FB)
            pd = psum.tile([DL, FB], F32, name=f"pd{b}", tag=f"pd{b}")
            pn = psum.tile([DL, FB], F32, name=f"pn{b}", tag=f"pn{b}")
            for j in range(3):
                nc.tensor.matmul(
                    out=pd, lhsT=ident_sb, rhs=sig[j][:, bsl],
                    start=(j == 0), stop=(j == 2),
                )
            for j in range(3):
                nc.tensor.matmul(
                    out=pn, lhsT=dg[j], rhs=sig[j][:, bsl],
                    start=(j == 0), stop=(j == 2),
                )
            de = temps.tile([DL, FB], F32, name=f"de{b}", tag=f"de{b}")
            nc.vector.tensor_scalar_add(out=de, in0=pd, scalar1=EPS_W)
            rr = temps.tile([DL, FB], F32, name=f"rr{b}", tag=f"rr{b}")
            nc.vector.reciprocal_approx_fast(out=rr, in_=de)
            # tt = num * r  (into de, in-place over dead den)
            nc.vector.tensor_mul(de, pn, rr)
            # out_un = (xv - mu_v) * tt
            nc.vector.scalar_tensor_tensor(
                out=ou[:, bsl], in0=x_sb["v"][:, i * F_main + b * FB:
                                              i * F_main + (b + 1) * FB],
                scalar=muv, in1=de,
                op0=Alu.subtract, op1=Alu.mult,
                accum_out=osum[:, 2 * i + b:2 * i + b + 1],
            )
        # sum of squares of out_un tile (ScalarE; main output is scratch)
        bG = temps.tile([DL, F_main], mybir.dt.bfloat16, name="bG", tag="bG")
        nc.scalar.activation(bG, ou, Act.Square, accum_out=osq[:, i:i + 1])
    return out_un


def build_program(reps=1, variant="base"):
    nc = bacc.Bacc("TRN2", num_devices=N_CORES)
    dram = {}
    for p in ("q", "k", "v"):
        dram["x" + p] = nc.dram_tensor("x" + p, [DL, BT], F32, kind="ExternalInput").ap()
        dram["u" + p] = nc.dram_tensor("u" + p, [DL, 3], F32, kind="ExternalInput").ap()
        dram["ug" + p] = nc.dram_tensor("ug" + p, [DL, 3], F32, kind="ExternalInput").ap()
    dram["g_out"] = nc.dram_tensor("g_out", [DL, 1], F32, kind="ExternalInput").ap()
    dram["b_out"] = nc.dram_tensor("b_out", [DL, 1], F32, kind="ExternalInput").ap()
    dram["ident"] = nc.dram_tensor("ident", [DL, DL], F32, kind="ExternalInput").ap()
    dram["out"] = nc.dram_tensor("out", [DL, BT], F32, kind="ExternalOutput").ap()

    with tile.TileContext(nc) as tc:
        for _ in range(reps):
            _emit_body(nc, tc, dram, variant=variant)
    nc.compile()
    return nc


def _softplus(x):
    return np.log1p(np.exp(-np.abs(x))) + np.maximum(x, 0.0)


def _host_params(w, b, a, g, beta):
    """Return (u, u*g) per channel (bias b cancels through the mean)."""
    Q = np.linalg.qr(np.asarray(a, dtype=np.float64))[0].astype(np.float32)
    u = np.einsum("di,dij->dj", _softplus(np.asarray(w, np.float64)).astype(np.float32), Q)
    return u, u * np.asarray(g, np.float32)


def _reference_fallback(x, wq, bq, aq, gq, betaq, wk, bk, ak, gk, betak,
                        wv, bv, av, gv, betav, g_out, b_out):
    """General-path numpy fallback (only used if some beta is nonzero)."""
    def block(xi, w, b, a, g, beta):
        h = xi[..., None] * _softplus(w) + b
        Q = np.linalg.qr(a)[0]
        h = np.einsum("btdi,dij->btdj", h, Q)
        mean = h.mean(axis=(0, 1))
        var = h.var(axis=(0, 1))
        return (h - mean) / np.sqrt(var + EPS_NORM) * g + beta

    d = D
    Qp = block(x[..., :d], wq, bq, aq, gq, betaq)
    Kp = block(x[..., d:2 * d], wk, bk, ak, gk, betak)
    Vp = block(x[..., 2 * d:], wv, bv, av, gv, betav)
    scores = 1.0 / (1.0 + np.exp(-GAMMA * (Qp * Kp)))
    weights = scores / (scores.sum(axis=-1, keepdims=True) + EPS_W)
    out = (weights * Vp).sum(axis=-1)
    mean = out.mean(axis=(0, 1))
    var = out.var(axis=(0, 1))
    return ((out - mean) / np.sqrt(var + EPS_NORM) * g_out + b_out).astype(np.float32)


_NC_CACHE = {}

VARIANT = "base"


def _get_program(reps=1, variant=None):
    if variant is None:
        variant = VARIANT
    key = (reps, variant)
    if key not in _NC_CACHE:
        _NC_CACHE[key] = build_program(reps, variant)
    return _NC_CACHE[key]


def _make_in_maps(x, params):
    """params: dict p -> (u, ug) full (D,3); x: (B,T,3D). Returns per-core maps."""
    x2 = np.asarray(x, np.float32).reshape(BT, 3 * D)
    # one-pass transpose into (24 blocks, DL channels, BT) channel-major
    xt = np.ascontiguousarray(
        x2.reshape(BT, 3 * N_CORES, DL).transpose(1, 2, 0))
    in_maps = []
    for c in range(N_CORES):
        m = {}
        for pi, p in enumerate(("q", "k", "v")):
            m["x" + p] = xt[pi * N_CORES + c]
            u, ug = params[p]
            m["u" + p] = np.ascontiguousarray(u[c * DL:(c + 1) * DL])
            m["ug" + p] = np.ascontiguousarray(ug[c * DL:(c + 1) * DL])
        m["g_out"] = np.ascontiguousarray(params["g_out"][c * DL:(c + 1) * DL, None])
        m["b_out"] = np.ascontiguousarray(params["b_out"][c * DL:(c + 1) * DL, None])
        m["ident"] = np.eye(DL, dtype=np.float32)
        in_maps.append(m)
    return in_maps


def kernel(x, wq, bq, aq, gq, betaq, wk, bk, ak, gk, betak,
           wv, bv, av, gv, betav, g_out, b_out):
    if (np.any(np.asarray(betaq)) or np.any(np.asarray(betak))
            or np.any(np.asarray(betav))):
        return _reference_fallback(x, wq, bq, aq, gq, betaq, wk, bk, ak, gk,
                                   betak, wv, bv, av, gv, betav, g_out, b_out)

    params = {
        "q": _host_params(wq, bq, aq, gq, betaq),
        "k": _host_params(wk, bk, ak, gk, betak),
        "v": _host_params(wv, bv, av, gv, betav),
        "g_out": np.asarray(g_out, np.float32),
        "b_out": np.asarray(b_out, np.float32),
    }
    nc = _get_program()
    in_maps = _make_in_maps(x, params)
    try:
        per_core = _run_cached(nc, in_maps)
    except Exception:
        res = bass_utils.run_bass_kernel_spmd(
            nc, in_maps, core_ids=list(range(N_CORES)))
        per_core = [res.results[c]["out"] for c in range(N_CORES)]
    out = np.empty((BT, D), np.float32)
    for c in range(N_CORES):
        out[:, c * DL:(c + 1) * DL] = per_core[c].T
    return out.reshape(B, T, D)


_RUNNER_CACHE = {}


def _run_cached(nc, in_maps):
    """Jit the bass_exec shard_map once; later kernel() calls only restage
    inputs (saves ~1-2 s of retracing/recompiling per call)."""
    key = id(nc)
    if key not in _RUNNER_CACHE:
        import jax
        from jax.sharding import Mesh, PartitionSpec, NamedSharding
        try:
            from jax import shard_map
        except ImportError:
            from jax.experimental.shard_map import shard_map
        from concourse import mybir as _mb
        from concourse.bass2jax import (
            _bass_exec_p, install_neuronx_cc_hook, partition_id_tensor)

        install_neuronx_cc_hook()
        pname = nc.partition_id_tensor.name if nc.partition_id_tensor else None
        in_names, out_names, out_avals, zero_outs = [], [], [], []
        for alloc in nc.m.functions[0].allocations:
            if not isinstance(alloc, _mb.MemoryLocationSet):
                continue
            name = alloc.memorylocations[0].name
            if alloc.kind == "ExternalInput":
                if name != pname:
                    in_names.append(name)
            elif alloc.kind == "ExternalOutput":
                out_names.append(name)
                shp = tuple(alloc.tensor_shape)
                dt_np = _mb.dt.np(alloc.dtype)
                out_avals.append(jax.core.ShapedArray(shp, dt_np))
                zero_outs.append(np.zeros(shp, dt_np))
        all_in = list(in_names) + list(out_names)
        if pname is not None:
            all_in.append(pname)

        def _body(*args):
            operands = list(args)
            if pname is not None:
                operands.append(partition_id_tensor())
            return tuple(_bass_exec_p.bind(
                *operands, out_avals=tuple(out_avals), in_names=tuple(all_in),
                out_names=tuple(out_names), lowering_input_output_aliases=(),
                sim_require_finite=True, sim_require_nnan=True, nc=nc))

        devices = jax.devices()[:N_CORES]
        mesh = Mesh(np.asarray(devices), ("core",))
        nspec = (PartitionSpec("core"),) * (len(in_names) + len(out_names))
        jitted = jax.jit(
            shard_map(_body, mesh=mesh, in_specs=nspec,
                      out_specs=(PartitionSpec("core"),) * len(out_names),
                      check_rep=False),
            keep_unused=True)
        sh = NamedSharding(mesh, PartitionSpec("core"))
        zconcat = [
            jax.device_put(
                np.zeros((N_CORES * z.shape[0], *z.shape[1:]), z.dtype), sh)
            for z in zero_outs]
        _RUNNER_CACHE[key] = (jitted, in_names, out_names, out_avals, sh, zconcat)
    import jax
    jitted, in_names, out_names, out_avals, sh, zconcat = _RUNNER_CACHE[key]
    args = [
        jax.device_put(
            np.concatenate([in_maps[c][nm] for c in range(N_CORES)], axis=0), sh)
        for nm in in_names]
    outs = jitted(*args, *zconcat)
    oi = out_names.index("out")
    full = np.asarray(outs[oi]).reshape(N_CORES, *out_avals[oi].shape)
    return [full[c] for c in range(N_CORES)]

